# revision 12
# baseline (speedup 1.0000x reference)
"""Trainium2 Bass kernel for nn_ConceptLayer (sparsemax + top-8 concept layer).

Computes, per token t (row of h):
    logits = h @ We.T + be                      # [*, 64]
    p      = sparsemax(logits)                  # entmax15() in the reference
    c      = top8_sparsify(p)                   # keep top-8 values in place
    h_out  = c @ Wd.T + bd                      # alpha = 1.0 -> h_out == h_rec

Sharding: pure data parallel over tokens (16*4096 = 65536 tokens) across 8
NeuronCores; tiny weights replicated.

Per-core dataflow (tokens tiled 128 at a time, 4 tiles per DMA macro-chunk):
  - PE transposes each [128, 128] h chunk (fp32 has no DMA-transpose path),
    encoder matmul accumulates logits [128tok, 64] in PSUM.
  - sparsemax via sorted top-16 (nc.vector.max / match_replace / max gives the
    16 largest, descending; measured max support size is 9), cumsum via
    tensor_tensor_scan, support/tau via fused scalar_tensor_tensor ops.
  - top-8 mask is `z >= z_sorted[7]` (bit-exact compare against the 8th
    largest logit -- monotone-equivalent to masking p).
  - decoder: PE-transpose concepts (augmented with a ones column so the
    matmul adds bd via an extra contraction row), 2 matmuls of N=384.
"""

import os
import sys

import numpy as np

for _p in ("/opt/trn_rl_repo", "/root/.axon_site/_ro/trn_rl_repo"):
    if os.path.isdir(_p) and _p not in sys.path:
        sys.path.insert(0, _p)

import concourse.bass as bass
import concourse.bacc as bacc
import concourse.tile as tile
from concourse import mybir
from concourse.bass_utils import run_bass_kernel_spmd
from concourse.masks import make_identity

F32 = mybir.dt.float32
OP = mybir.AluOpType
ACTF = mybir.ActivationFunctionType

N_CORES = 8
B, S, D, M = 16, 4096, 768, 64
TOK = B * S
P = 128
DC = D // P          # 6 d-chunks of 128
G = 4                # token-tiles per DMA macro-chunk
NEG = -1.0e30


def _broadcast_ap(src: bass.AP, parts: int) -> bass.AP:
    """View a 1-D DRAM tensor as [parts, n] with 0-stride partition dim."""
    return bass.AP(tensor=src.tensor, offset=src.offset, ap=[[0, parts]] + list(src.ap))


def _kernel_body(tc: tile.TileContext, ctx, h, We, be, Wd, bd, hout, cout, tok_per_core,
                 stage: int = 9):
    nc = tc.nc
    tiles = tok_per_core // P
    chunks = tiles // G

    consts = ctx.enter_context(tc.tile_pool(name="consts", bufs=1))

    ident = consts.tile([P, P], F32)
    make_identity(nc, ident)

    # rho = [1..16] broadcast over partitions; ones for the cumsum scan.
    rho_i = consts.tile([P, 16], mybir.dt.int32)
    nc.gpsimd.iota(rho_i, pattern=[[1, 16]], base=1, channel_multiplier=0)
    rho = consts.tile([P, 16], F32)
    nc.vector.tensor_copy(rho, rho_i)
    ones16 = consts.tile([P, 16], F32)
    nc.vector.memset(ones16, 1.0)
    ones1 = ones16[:, 0:1]

    # be broadcast to [P, M] (added to logits during the PSUM->SBUF copy).
    be_b = consts.tile([P, M], F32)
    nc.gpsimd.dma_start(out=be_b, in_=_broadcast_ap(be, P))

    # Encoder weights: We [M, D] -> WeT chunks [P, DC, M] (WeT[d, c] = We[c, d]).
    # Decoder weights: Wd [D, M] -> WdT_aug [M+1, D]; row M = bd so the ones
    # column of the augmented concepts adds the bias.
    we_sb = consts.tile([M, D], F32)
    nc.sync.dma_start(out=we_sb, in_=We)
    wet = consts.tile([P, DC, M], F32)
    wd_view = Wd.rearrange("(j p) m -> p j m", p=P)
    wd_sb = consts.tile([P, DC, M], F32)
    nc.sync.dma_start(out=wd_sb, in_=wd_view)
    wdt = consts.tile([M + 1, D], F32)
    with tc.tile_pool(name="ps_setup", bufs=2, space="PSUM") as ps_setup:
        for j in range(DC):
            pt = ps_setup.tile([P, M], F32, tag="setup")
            nc.tensor.transpose(pt, we_sb[:, j * P : (j + 1) * P], ident[0:M, 0:M])
            nc.scalar.copy(out=wet[:, j, :], in_=pt)
        for j in range(DC):
            pt = ps_setup.tile([M, P], F32, tag="setup")
            nc.tensor.transpose(pt, wd_sb[:, j, :], ident)
            nc.scalar.copy(out=wdt[0:M, j * P : (j + 1) * P], in_=pt)
    nc.gpsimd.dma_start(out=wdt[M : M + 1, :], in_=_broadcast_ap(bd, 1))

    # Streaming pools.
    h_pool = ctx.enter_context(tc.tile_pool(name="h_in", bufs=3))
    ht_pool = ctx.enter_context(tc.tile_pool(name="ht", bufs=2))
    mid = ctx.enter_context(tc.tile_pool(name="mid", bufs=2))
    ct_pool = ctx.enter_context(tc.tile_pool(name="ct", bufs=2))
    out_pool = ctx.enter_context(tc.tile_pool(name="out", bufs=2))
    ps_ht = ctx.enter_context(tc.tile_pool(name="ps_ht", bufs=3, space="PSUM"))
    ps_lg = ctx.enter_context(tc.tile_pool(name="ps_lg", bufs=2, space="PSUM"))
    ps_ct = ctx.enter_context(tc.tile_pool(name="ps_ct", bufs=1, space="PSUM"))
    ps_hr = ctx.enter_context(tc.tile_pool(name="ps_hr", bufs=2, space="PSUM"))

    hv = h.rearrange("(c g p) d -> c p g d", p=P, g=G)
    hov = hout.rearrange("(c g p) d -> c p g d", p=P, g=G)
    cov = cout.rearrange("(c g p) m -> c p g m", p=P, g=G)

    for c in range(chunks):
        h_sb = h_pool.tile([P, G, D], F32, tag="h")
        nc.sync.dma_start(out=h_sb, in_=hv[c])
        hrec_sb = out_pool.tile([P, G * D], F32, tag="hrec")
        conc_sb = out_pool.tile([P, G, M + 1], F32, tag="conc")
        for g in range(G):
            # --- encoder: transpose h tile, matmul against WeT ---
            ht_ps0 = ps_ht.tile([P, 3 * P], F32, tag="htp")
            ht_ps1 = ps_ht.tile([P, 3 * P], F32, tag="htp")
            for j in range(DC):
                tgt = ht_ps0 if j < 3 else ht_ps1
                nc.tensor.transpose(
                    tgt[:, (j % 3) * P : (j % 3 + 1) * P],
                    h_sb[:, g, j * P : (j + 1) * P],
                    ident,
                )
            ht_sb = ht_pool.tile([P, DC * P], F32, tag="ht")
            nc.scalar.copy(out=ht_sb[:, 0 : 3 * P], in_=ht_ps0)
            nc.scalar.copy(out=ht_sb[:, 3 * P : 6 * P], in_=ht_ps1)

            lg_ps = ps_lg.tile([P, M], F32, tag="lg")
            for j in range(DC):
                nc.tensor.matmul(
                    lg_ps,
                    lhsT=ht_sb[:, j * P : (j + 1) * P],
                    rhs=wet[:, j, :],
                    start=(j == 0),
                    stop=(j == DC - 1),
                )

            # z = logits + be  (PSUM -> SBUF)
            z = mid.tile([P, M], F32, tag="z")
            nc.vector.scalar_tensor_tensor(
                out=z, in0=lg_ps, scalar=0.0, in1=be_b, op0=OP.add, op1=OP.add
            )

            if stage <= 2:
                nc.vector.tensor_copy(out=conc_sb[:, g, 0:M], in_=z)
                nc.vector.memset(conc_sb[:, g, M:M+1], 1.0)
                nc.scalar.copy(out=hrec_sb[:, g * D : (g + 1) * D], in_=h_sb[:, g, :])
                continue

            # --- sparsemax: sorted top-16, cumsum, support, tau ---
            s16 = mid.tile([P, 16], F32, tag="s16")
            z2 = mid.tile([P, M], F32, tag="z2")
            nc.vector.max(out=s16[:, 0:8], in_=z)
            nc.vector.match_replace(
                out=z2, in_to_replace=s16[:, 0:8], in_values=z, imm_value=NEG
            )
            nc.vector.max(out=s16[:, 8:16], in_=z2)

            if stage <= 3:
                nc.vector.tensor_copy(out=conc_sb[:, g, 0:16], in_=s16)
                nc.vector.tensor_copy(out=conc_sb[:, g, 16:M], in_=z[:, 16:M])
                nc.vector.memset(conc_sb[:, g, M:M+1], 1.0)
                nc.scalar.copy(out=hrec_sb[:, g * D : (g + 1) * D], in_=h_sb[:, g, :])
                continue

            cs = mid.tile([P, 16], F32, tag="cs")
            nc.vector.tensor_tensor_scan(
                out=cs, data0=ones16, data1=s16, initial=0.0, op0=OP.mult, op1=OP.add
            )
            t1 = mid.tile([P, 16], F32, tag="t1")
            nc.vector.tensor_tensor(out=t1, in0=s16, in1=rho, op=OP.mult)
            sml = mid.tile([P, 8], F32, tag="sml")
            k_ap, kinv_ap, ssum_ap, taun_ap = (sml[:, i : i + 1] for i in range(4))
            psum_ap, pinv_ap = sml[:, 4:5], sml[:, 5:6]
            s_ind = mid.tile([P, 16], F32, tag="sind")
            # support_j = (z_sorted_j * rho_j + 1) > cumsum_j ; k = sum(support)
            nc.vector.scalar_tensor_tensor(
                out=s_ind, in0=t1, scalar=1.0, in1=cs,
                op0=OP.add, op1=OP.is_gt, accum_out=k_ap,
            )
            # ssum = sum(z_sorted * support)  (accum_out does the reduce)
            nc.vector.scalar_tensor_tensor(
                out=t1, in0=s16, scalar=0.0, in1=s_ind,
                op0=OP.add, op1=OP.mult, accum_out=ssum_ap,
            )
            nc.vector.reciprocal(out=kinv_ap, in_=k_ap)
            # tau_neg = (1 - ssum) / k
            nc.vector.scalar_tensor_tensor(
                out=taun_ap, in0=ssum_ap, scalar=-1.0, in1=ones1,
                op0=OP.mult, op1=OP.add,
            )
            nc.vector.tensor_scalar(
                out=taun_ap, in0=taun_ap, scalar1=kinv_ap, scalar2=None, op0=OP.mult
            )

            if stage <= 4:
                nc.vector.tensor_copy(out=conc_sb[:, g, 0:8], in_=sml)
                nc.vector.tensor_copy(out=conc_sb[:, g, 8:M], in_=z[:, 8:M])
                nc.vector.memset(conc_sb[:, g, M:M+1], 1.0)
                nc.scalar.copy(out=hrec_sb[:, g * D : (g + 1) * D], in_=h_sb[:, g, :])
                continue

            # p = relu(z + tau_neg); psum accumulated by the same ACT op
            p_t = mid.tile([P, M], F32, tag="p")
            nc.scalar.activation(
                out=p_t, in_=z, func=ACTF.Relu, bias=taun_ap, scale=1.0,
                accum_out=psum_ap,
            )
            nc.vector.reciprocal(out=pinv_ap, in_=psum_ap)

            # concepts = (z >= z_sorted[7]) * p / psum ; ones column for bias
            mask = mid.tile([P, M], F32, tag="mask")
            nc.vector.tensor_scalar(
                out=mask, in0=z, scalar1=s16[:, 7:8], scalar2=None, op0=OP.is_ge
            )
            nc.vector.scalar_tensor_tensor(
                out=conc_sb[:, g, 0:M], in0=p_t, scalar=pinv_ap, in1=mask,
                op0=OP.mult, op1=OP.mult,
            )
            nc.vector.memset(conc_sb[:, g, M : M + 1], 1.0)

            if stage <= 5:
                nc.scalar.copy(out=hrec_sb[:, g * D : (g + 1) * D], in_=h_sb[:, g, :])
                continue

            # --- decoder ---
            ct_ps = ps_ct.tile([M + 1, P], F32, tag="ct")
            nc.tensor.transpose(ct_ps, conc_sb[:, g, :], ident)
            ct_sb = ct_pool.tile([M + 1, P], F32, tag="ct")
            nc.scalar.copy(out=ct_sb, in_=ct_ps)
            hr_ps0 = ps_hr.tile([P, 384], F32, tag="hr")
            hr_ps1 = ps_hr.tile([P, 384], F32, tag="hr")
            nc.tensor.matmul(hr_ps0, lhsT=ct_sb, rhs=wdt[:, 0:384], start=True, stop=True)
            nc.tensor.matmul(hr_ps1, lhsT=ct_sb, rhs=wdt[:, 384:768], start=True, stop=True)
            nc.scalar.copy(out=hrec_sb[:, g * D : g * D + 384], in_=hr_ps0)
            nc.vector.tensor_copy(out=hrec_sb[:, g * D + 384 : (g + 1) * D], in_=hr_ps1)

        nc.sync.dma_start(out=hov[c], in_=hrec_sb.rearrange("p (g d) -> p g d", g=G))
        nc.sync.dma_start(out=cov[c], in_=conc_sb[:, :, 0:M])


def build_nc(tok_per_core: int, stage: int = 9) -> bass.Bass:
    from contextlib import ExitStack

    nc = bacc.Bacc()
    h = nc.dram_tensor("h", [tok_per_core, D], F32, kind="ExternalInput")
    We = nc.dram_tensor("We", [M, D], F32, kind="ExternalInput")
    be = nc.dram_tensor("be", [M], F32, kind="ExternalInput")
    Wd = nc.dram_tensor("Wd", [D, M], F32, kind="ExternalInput")
    bd = nc.dram_tensor("bd", [D], F32, kind="ExternalInput")
    hout = nc.dram_tensor("h_out", [tok_per_core, D], F32, kind="ExternalOutput")
    cout = nc.dram_tensor("concepts", [tok_per_core, M], F32, kind="ExternalOutput")
    with tile.TileContext(nc) as tc:
        with ExitStack() as ctx:
            _kernel_body(
                tc, ctx, h[:], We[:], be[:], Wd[:], bd[:], hout[:], cout[:],
                tok_per_core, stage=stage,
            )
    nc.finalize()  # Bacc: legalize multi-sem waits (event sems), alloc regs
    return nc


_NC_CACHE: dict[int, bass.Bass] = {}
LAST_RESULTS = None  # BassKernelResults of the most recent kernel() call


def kernel(h, We, be, Wd, bd, **run_kwargs):
    global LAST_RESULTS
    h = np.ascontiguousarray(np.asarray(h, dtype=np.float32))
    We = np.ascontiguousarray(np.asarray(We, dtype=np.float32))
    be = np.ascontiguousarray(np.asarray(be, dtype=np.float32))
    Wd = np.ascontiguousarray(np.asarray(Wd, dtype=np.float32))
    bd = np.ascontiguousarray(np.asarray(bd, dtype=np.float32))

    b, s, d = h.shape
    tok = b * s
    tok_per_core = tok // N_CORES
    hf = h.reshape(tok, d)

    if tok_per_core not in _NC_CACHE:
        _NC_CACHE[tok_per_core] = build_nc(tok_per_core)
    nc = _NC_CACHE[tok_per_core]

    in_maps = [
        {
            "h": hf[i * tok_per_core : (i + 1) * tok_per_core],
            "We": We,
            "be": be,
            "Wd": Wd,
            "bd": bd,
        }
        for i in range(N_CORES)
    ]
    res = run_bass_kernel_spmd(nc, in_maps, core_ids=list(range(N_CORES)), **run_kwargs)
    LAST_RESULTS = res
    h_out = np.concatenate([r["h_out"] for r in res.results], axis=0).reshape(b, s, d)
    concepts = np.concatenate([r["concepts"] for r in res.results], axis=0).reshape(b, s, M)
    return h_out, concepts
